# revision 1
# baseline (speedup 1.0000x reference)
"""Trainium2 Bass kernel v5: Wiener deconvolution via 4-step CT FFT matmuls.

v5 over v4: H-twiddle split DVE(comp0,psum-direct)/Pool(comp1,ACT-staged);
x-twiddle on DVE psum-direct (no staging); PE warm-fill matmuls bridge the
forward gap so M2 runs at full p-state; finer PSUM pool lifetimes (Hps reuses
Ah's banks); a quarter of evac1s on DVE; per-half G broadcasts on SP.
"""
import sys

sys.path.insert(0, "/opt/trn_rl_repo")

import numpy as np


def _get_cc():
    import concourse.bacc as bacc
    import concourse.mybir as mybir
    import concourse.tile as tile
    return bacc, mybir, tile


class Cfg:
    def __init__(self, T=8192, N2=128, N1=64, BL=2, C=8, FIL=16):
        assert N1 * N2 == T
        self.T, self.N2, self.N1, self.BL, self.C, self.FIL = T, N2, N1, BL, C, FIL
        self.ROWS = BL * C
        self.FC = FIL * C


FULL = Cfg()

POOL_PAIRS = {(1, 0), (1, 4), (1, 8), (1, 12), (0, 4), (0, 8), (0, 12)}
EVAC1_DVE = lambda b, f: f in (2, 7, 12)


def host_consts(cfg):
    T, N1, N2 = cfg.T, cfg.N1, cfg.N2
    f32 = np.float32
    a2, a1 = np.arange(N2), np.arange(N1)
    cs = {}
    F2 = np.exp(-2j * np.pi * np.outer(a2, a2) / N2)          # [n2,k2]
    cs["blob_r"] = np.concatenate(
        [F2.real, F2.imag, -F2.imag], axis=1).astype(f32)
    Tw = np.exp(-2j * np.pi * np.outer(a2, a1) / T)           # [k2,n1]
    brep_placeholder = np.zeros((N2, cfg.FC), f32)
    cs["blob_f"] = np.concatenate(
        [Tw.real, Tw.imag, -Tw.imag, brep_placeholder], axis=1).astype(f32)
    F1 = np.exp(-2j * np.pi * np.outer(a1, a1) / N1)          # [n1,k1]
    M2 = np.hstack([np.vstack([F1.real, -F1.imag]),
                    np.vstack([F1.imag, F1.real])]).astype(f32)
    Fb1 = np.exp(2j * np.pi * np.outer(a1, a1) / N1)          # [k1,n1']
    M3 = np.hstack([np.vstack([Fb1.real, -Fb1.imag]),
                    np.vstack([Fb1.imag, Fb1.real])]).astype(f32)
    M3sw = np.vstack([-M3[N1:], M3[:N1]]).astype(f32)
    pad = np.ones((2 * N1, 1), f32)
    cs["blob_b"] = np.concatenate([M2, M3, M3sw, pad], axis=1).astype(f32)
    L = np.exp(2j * np.pi * (np.outer(a2, a2)[None, :, :] / N2
                             + (a1[:, None, None] * a2[None, :, None]) / T)) / T
    cL = np.empty((N2, 2, N1, N2), f32)                        # [k2, ri, n1', n2']
    cL[:, 0] = L.real.transpose(1, 0, 2)
    cL[:, 1] = -L.imag.transpose(1, 0, 2)
    cs["c_L"] = cL.reshape(N2, 2 * N1 * N2)
    return cs


def build_nc(cfg):
    bacc, mybir, tile = _get_cc()
    F32, F32R, BF16 = mybir.dt.float32, mybir.dt.float32r, mybir.dt.bfloat16
    AL = mybir.AluOpType
    T, N1, N2, BL, C, FIL = cfg.T, cfg.N2 * cfg.N1, cfg.N2, cfg.N1, cfg.BL, cfg.C
    # (re-bind clean)
    T, N2, N1, BL, C, FIL = cfg.T, cfg.N2, cfg.N1, cfg.BL, cfg.C, cfg.FIL
    FC = cfg.FC
    N1s = 2 * N1
    KF = FIL * N2
    MCH = 512
    HN = FIL * N1      # 1024
    XNb = N1 * C       # 512
    XN = BL * XNb      # 1024
    CK = C * N2        # 1024

    nc = bacc.Bacc("TRN2", debug=False)

    xs_d = nc.dram_tensor("xs", [N2, XN], F32R, kind="ExternalInput")
    wr_d = nc.dram_tensor("wr", [N2, HN], F32R, kind="ExternalInput")
    wi_d = nc.dram_tensor("wi", [N2, HN], F32R, kind="ExternalInput")
    s64_d = nc.dram_tensor("s64", [N1, FIL], F32, kind="ExternalInput")
    brow_d = nc.dram_tensor("brow", [1, FC], BF16, kind="ExternalInput")
    onesr_d = nc.dram_tensor("onesr", [1, N2], BF16, kind="ExternalInput")
    blob_r_d = nc.dram_tensor("blob_r", [N2, 3 * N2], F32R, kind="ExternalInput")
    blob_f_d = nc.dram_tensor("blob_f", [N2, 3 * N1 + FC], F32, kind="ExternalInput")
    blob_b_d = nc.dram_tensor("blob_b", [N1s, 3 * N1s + 1], BF16, kind="ExternalInput")
    cL_d = nc.dram_tensor("c_L", [N2, 2 * N1 * N2], BF16, kind="ExternalInput")
    out_d = nc.dram_tensor("out", [BL, T, FC], BF16, kind="ExternalOutput")

    def chunks(total):
        return [(c0, min(total, c0 + MCH)) for c0 in range(0, total, MCH)]

    with tile.TileContext(nc) as tc:
        from contextlib import ExitStack
        with tc.tile_pool(name="consts", bufs=1) as cpool, \
             tc.tile_pool(name="pers", bufs=1) as pers:
            blob_r = cpool.tile([N2, 3 * N2], F32R, tag="blob_r")
            nc.sync.dma_start(out=blob_r, in_=blob_r_d.ap())
            wtr = cpool.tile([N2, HN], F32R, tag="wtr")
            nc.sync.dma_start(out=wtr, in_=wr_d.ap())
            wti = cpool.tile([N2, HN], F32R, tag="wti")
            nc.sync.dma_start(out=wti, in_=wi_d.ap())
            xt = cpool.tile([N2, XN], F32R, tag="xt")
            nc.sync.dma_start(out=xt, in_=xs_d.ap())
            blob_f = cpool.tile([N2, 3 * N1 + FC], F32, tag="blob_f")
            nc.sync.dma_start(out=blob_f, in_=blob_f_d.ap())
            blob_b = cpool.tile([N1s, 3 * N1s + 1], BF16, tag="blob_b")
            nc.sync.dma_start(out=blob_b, in_=blob_b_d.ap())
            s64 = cpool.tile([N1, FIL], F32, tag="s64")
            nc.sync.dma_start(out=s64, in_=s64_d.ap())
            brow = cpool.tile([1, FC], BF16, tag="brow")
            nc.sync.dma_start(out=brow, in_=brow_d.ap())
            onesr = cpool.tile([1, N2], BF16, tag="onesr")
            nc.sync.dma_start(out=onesr, in_=onesr_d.ap())
            cL = cpool.tile([N2, 2 * N1 * N2], BF16, tag="cL")
            nc.sync.dma_start(out=cL, in_=cL_d.ap())

            F2r = blob_r[:, 0:N2]
            F2i = blob_r[:, N2:2 * N2]
            F2in = blob_r[:, 2 * N2:3 * N2]
            Twr = blob_f[:, 0:N1]
            Twi = blob_f[:, N1:2 * N1]
            Twin = blob_f[:, 2 * N1:3 * N1]
            brep = blob_f[:, 3 * N1:3 * N1 + FC]
            cM2 = blob_b[:, 0:N1s]
            cM3 = blob_b[:, N1s:2 * N1s]
            cM3sw = blob_b[:, 2 * N1s:3 * N1s]

            Z0A = pers.tile([N1s, XN * 2], BF16, tag="Z0A")        # [k1s | (b,c,k2)]
            sqB = pers.tile([N1, FIL * N2], F32, tag="sqB")
            G1 = pers.tile([N1s, KF], BF16, tag="G1")
            G2 = pers.tile([N1s, KF], BF16, tag="G2")

            fes = ExitStack()
            fwd = fes.enter_context(tc.tile_pool(name="fwd", bufs=1))
            pxes = ExitStack()
            pAx = pxes.enter_context(tc.tile_pool(name="pAx", bufs=1, space="PSUM"))
            phes = ExitStack()
            pAh = phes.enter_context(tc.tile_pool(name="pAh", bufs=1, space="PSUM"))

            # ---------- M1 H ----------
            Ah = pAh.tile([N2, 2 * HN], F32, tag="Ah")              # [k2 | (comp,f,n1)]
            for c0, c1 in chunks(HN):
                nc.tensor.matmul(Ah[:, c0:c1], F2r, wtr[:, c0:c1], start=True, stop=False)
                nc.tensor.matmul(Ah[:, c0:c1], F2in, wti[:, c0:c1], start=False, stop=True)
                nc.tensor.matmul(Ah[:, HN + c0:HN + c1], F2i, wtr[:, c0:c1],
                                 start=True, stop=False)
                nc.tensor.matmul(Ah[:, HN + c0:HN + c1], F2r, wti[:, c0:c1],
                                 start=False, stop=True)

            # ---------- M1 x ----------
            Ax = pAx.tile([N2, 2 * XN], F32, tag="Ax")              # [k2 | (comp,b,n1,c)]
            for c0, c1 in chunks(XN):
                nc.tensor.matmul(Ax[:, c0:c1], F2r, xt[:, c0:c1], start=True, stop=True)
                nc.tensor.matmul(Ax[:, XN + c0:XN + c1], F2i, xt[:, c0:c1],
                                 start=True, stop=True)

            # ---------- H twiddle: comp0 on DVE (psum-direct), comp1 on Pool ----------
            Bh = fwd.tile([N2, FIL * 2 * N1], BF16, tag="Bh")      # [k2 | (f,comp,n1)]
            Bhv = Bh.rearrange("p (f m n) -> p f m n", f=FIL, m=2)
            Ahr = Ah[:, :HN].rearrange("p (f n) -> p f n", f=FIL)
            Ahi = Ah[:, HN:].rearrange("p (f n) -> p f n", f=FIL)
            # stage Ah to SBUF for the Pool half
            Ahs = fwd.tile([N2, 2 * HN], F32, tag="Ahs")
            nc.scalar.copy(out=Ahs[:, :HN], in_=Ah[:, :HN])
            nc.scalar.copy(out=Ahs[:, HN:], in_=Ah[:, HN:])
            Asr = Ahs[:, :HN].rearrange("p (f n) -> p f n", f=FIL)
            Asi = Ahs[:, HN:].rearrange("p (f n) -> p f n", f=FIL)
            uh = fwd.tile([N2, HN], F32, tag="uh")
            vh = fwd.tile([N2, HN], F32, tag="vh")
            uhv = uh.rearrange("p (f n) -> p f n", f=FIL)
            vhv = vh.rearrange("p (f n) -> p f n", f=FIL)
            uh2 = fwd.tile([N2, HN], F32, tag="uh2")
            vh2 = fwd.tile([N2, HN], F32, tag="vh2")
            uh2v = uh2.rearrange("p (f n) -> p f n", f=FIL)
            vh2v = vh2.rearrange("p (f n) -> p f n", f=FIL)

            def bch(w):
                return w[:, None, :].broadcast_to([N2, FIL, N1])

            nc.vector.tensor_tensor(out=uhv, in0=Ahr, in1=bch(Twr), op=AL.mult)
            nc.vector.tensor_tensor(out=vhv, in0=Ahi, in1=bch(Twin), op=AL.mult)
            nc.vector.tensor_tensor(out=Bhv[:, :, 0, :], in0=uhv, in1=vhv, op=AL.add)
            nc.gpsimd.tensor_tensor(out=uh2v, in0=Asr, in1=bch(Twi), op=AL.mult)
            nc.gpsimd.tensor_tensor(out=vh2v, in0=Asi, in1=bch(Twr), op=AL.mult)
            nc.gpsimd.tensor_tensor(out=Bhv[:, :, 1, :], in0=uh2v, in1=vh2v, op=AL.add)

            # ---------- x twiddle on DVE (psum-direct), b1 then b0 ----------
            Bc = fwd.tile([N2, BL * C * 2 * N1], BF16, tag="Bc")   # [k2|(b,c,comp,n1)]
            Bcv = Bc.rearrange("p (b c m n) -> p b c m n", b=BL, c=C, m=2)
            Axr = Ax[:, :XN].rearrange("p (b n c) -> p b c n", b=BL, c=C)
            Axi = Ax[:, XN:].rearrange("p (b n c) -> p b c n", b=BL, c=C)
            ux = fwd.tile([N2, XNb], F32, tag="ux")
            vx = fwd.tile([N2, XNb], F32, tag="vx")
            uxv = ux.rearrange("p (c n) -> p c n", c=C)
            vxv = vx.rearrange("p (c n) -> p c n", c=C)

            def bcx(w):
                return w[:, None, :].broadcast_to([N2, C, N1])

            for b in (1, 0):
                nc.vector.tensor_tensor(out=uxv, in0=Axr[:, b], in1=bcx(Twr), op=AL.mult)
                nc.vector.tensor_tensor(out=vxv, in0=Axi[:, b], in1=bcx(Twin), op=AL.mult)
                nc.vector.tensor_tensor(out=Bcv[:, b, :, 0, :], in0=uxv, in1=vxv, op=AL.add)
                nc.vector.tensor_tensor(out=uxv, in0=Axr[:, b], in1=bcx(Twi), op=AL.mult)
                nc.vector.tensor_tensor(out=vxv, in0=Axi[:, b], in1=bcx(Twr), op=AL.mult)
                nc.vector.tensor_tensor(out=Bcv[:, b, :, 1, :], in0=uxv, in1=vxv, op=AL.add)

            # ---------- T1 block transposes (DMA xbar) ----------
            BTH = fwd.tile([N1s, KF], BF16, tag="BTH")             # [(comp n1)|(f,k2)]
            nc.sync.dma_start_transpose(
                out=BTH.rearrange("p (f q) -> p f q", f=FIL), in_=Bh)
            BT = fwd.tile([N1s, XN * 2], BF16, tag="BT")           # [(comp n1)|(b,c,k2)]
            BTv = BT.rearrange("p (b c q) -> p b c q", b=BL, c=C)
            nc.sync.dma_start_transpose(out=BTv[:, 1], in_=Bc[:, CK:])
            nc.sync.dma_start_transpose(out=BTv[:, 0], in_=Bc[:, :CK])

            # ---------- M2h (Hps reuses Ah's banks only) ----------
            phes.close()
            pHes = ExitStack()
            pH = pHes.enter_context(tc.tile_pool(name="pH", bufs=1, space="PSUM"))
            Hps = pH.tile([N1s, KF], F32, tag="Hps")
            for c0, c1 in chunks(KF):
                nc.tensor.matmul(Hps[:, c0:c1], cM2, BTH[:, c0:c1], start=True, stop=True)
            Hs = pers.tile([N1s, KF], F32, tag="Hs")
            sq = pers.tile([N1s, KF], F32, tag="sq")
            HiB = pers.tile([N1, KF], F32, tag="HiB")
            QW = KF // 4
            for q in range(4):
                qs = slice(q * QW, (q + 1) * QW)
                nc.scalar.square(sq[:, qs], Hps[:, qs])
                nc.scalar.copy(out=sqB[:, qs], in_=sq[N1:, qs])
                nc.scalar.copy(out=Hs[:, qs], in_=Hps[:, qs])
                nc.scalar.copy(out=HiB[:, qs], in_=Hs[N1:, qs])

            # ---------- M2x ----------
            pHes.close()
            pxes.close()
            pZes = ExitStack()
            pZ = pZes.enter_context(tc.tile_pool(name="pZ", bufs=1, space="PSUM"))
            Zps = pZ.tile([N1s, XN * 2], F32, tag="Zps")           # [k1s | (b,c,k2)]
            for b in (1, 0):
                for c0, c1 in chunks(CK):
                    nc.tensor.matmul(Zps[:, b * CK + c0:b * CK + c1], cM2,
                                     BT[:, b * CK + c0:b * CK + c1], start=True, stop=True)
                nc.scalar.copy(out=Z0A[:, b * CK:(b + 1) * CK],
                               in_=Zps[:, b * CK:(b + 1) * CK])

            # ---------- G, pipelined per f-quarter (DVE); d in-place in sqB ----------
            def g_quarter(q):
                qs = slice(q * QW, (q + 1) * QW)
                nfq = FIL // 4
                sbv = sqB[:, qs].rearrange("p (f q) -> p f q", f=nfq)
                s64q = s64[:, q * nfq:(q + 1) * nfq, None].broadcast_to(
                    [N1, nfq, N2])
                nc.vector.tensor_tensor(out=sqB[:, qs], in0=sq[:N1, qs],
                                        in1=sqB[:, qs], op=AL.add)
                nc.vector.tensor_tensor(out=sbv, in0=sbv, in1=s64q, op=AL.add)
                nc.vector.reciprocal(out=sq[:N1, qs], in_=sqB[:, qs])
                nc.vector.tensor_tensor(out=G1[:N1, qs], in0=Hs[:N1, qs],
                                        in1=sq[:N1, qs], op=AL.mult)
                nc.vector.tensor_tensor(out=G2[:N1, qs], in0=HiB[:, qs],
                                        in1=sq[:N1, qs], op=AL.mult)
                nc.vector.tensor_copy(out=G1[N1:, qs], in_=G1[:N1, qs])
                nc.vector.tensor_copy(out=G2[N1:, qs], in_=G2[:N1, qs])

            g_quarter(0)

            pZes.close()
            fes.close()

            # ================= inverse =================
            with tc.tile_pool(name="dt", bufs=1) as dtp, \
                 tc.tile_pool(name="stg", bufs=1) as stp, \
                 tc.tile_pool(name="zt", bufs=3) as ztp, \
                 tc.tile_pool(name="cse", bufs=3) as csp, \
                 tc.tile_pool(name="ddp", bufs=2, space="PSUM") as ddp, \
                 tc.tile_pool(name="yp", bufs=4, space="PSUM") as yps:
                DT0 = dtp.tile([N2, N1s * FIL * C], BF16, tag="DT0")
                DT1 = dtp.tile([N2, N1s * FIL * C], BF16, tag="DT1")
                DT = [DT0, DT1]                                    # [k2 | (f,c,n1s')]
                STG0 = stp.tile([N2, N1 * FC], BF16, tag="STG0")
                STG1 = stp.tile([N2, N1 * FC], BF16, tag="STG1")
                STG = [STG0, STG1]                                 # [n2' | (n1',fc)]
                cLv = cL.rearrange("p (m n q) -> p m n q", m=2, n=N1)
                zvA = Z0A.rearrange("p (b c q) -> p b c q", b=BL, c=C)

                def cmul_m3_t2(b, f):
                    eng = nc.gpsimd if (b, f) in POOL_PAIRS else nc.vector
                    g1 = G1[:, f * N2:(f + 1) * N2][:, None, :].broadcast_to([N1s, C, N2])
                    g2 = G2[:, f * N2:(f + 1) * N2][:, None, :].broadcast_to([N1s, C, N2])
                    zt1 = ztp.tile([N1s, CK], BF16, tag="zt1")
                    zt2 = ztp.tile([N1s, CK], BF16, tag="zt2")
                    eng.tensor_tensor(out=zt1.rearrange("p (c q) -> p c q", c=C),
                                      in0=zvA[:, b], in1=g1, op=AL.mult)
                    eng.tensor_tensor(out=zt2.rearrange("p (c q) -> p c q", c=C),
                                      in0=zvA[:, b], in1=g2, op=AL.mult)
                    DD = ddp.tile([N1s, CK], F32, tag="DD")
                    for c0, c1 in chunks(CK):
                        nc.tensor.matmul(DD[:, c0:c1], cM3, zt1[:, c0:c1],
                                         start=True, stop=False)
                        nc.tensor.matmul(DD[:, c0:c1], cM3sw, zt2[:, c0:c1],
                                         start=False, stop=True)
                    cse = csp.tile([N1s, CK], BF16, tag="cse")
                    if EVAC1_DVE(b, f):
                        nc.vector.tensor_copy(out=cse, in_=DD)
                    else:
                        nc.scalar.copy(out=cse, in_=DD)
                    dtv = DT[b].rearrange("p (fi c n) -> p fi c n",
                                          n=N1s, fi=FIL, c=C)[:, f]
                    nc.sync.dma_start_transpose(out=dtv, in_=cse)

                def m4_mm(b, g0, gn=4, seeded=True):
                    dtm = DT[b].rearrange("p (fi c n) -> p n fi c", n=N1s, fi=FIL, c=C)
                    ypsum = yps.tile([N2, gn * FC], F32, tag="yps")
                    for j in range(gn):
                        n1p = g0 + j
                        sl = ypsum[:, j * FC:(j + 1) * FC]
                        if seeded:
                            nc.tensor.matmul(sl, onesr, brow, start=True, stop=False)
                        nc.tensor.matmul(sl, cLv[:, 0, n1p, :], dtm[:, n1p],
                                         start=not seeded, stop=False)
                        nc.tensor.matmul(sl, cLv[:, 1, n1p, :], dtm[:, N1 + n1p],
                                         start=False, stop=True)
                    return ypsum

                def m4_evac(b, g0, ypsum, gn=4, eng="act"):
                    dst = STG[b][:, g0 * FC:(g0 + gn) * FC]
                    if eng == "act":
                        nc.scalar.copy(out=dst, in_=ypsum)
                    else:
                        bb = brep[:, None, :].broadcast_to([N2, gn, FC])
                        nc.vector.tensor_tensor(
                            out=dst.rearrange("p (j fc) -> p j fc", j=gn),
                            in0=ypsum.rearrange("p (j fc) -> p j fc", j=gn),
                            in1=bb, op=AL.add)

                def m4_group(b, g0, gn=4, eng="act"):
                    seeded = eng == "act"
                    m4_evac(b, g0, m4_mm(b, g0, gn, seeded), gn, eng)

                def out_chunk(b, g0, gn=16):
                    nc.scalar.dma_start(
                        out=out_d.ap()[b].rearrange(
                            "(q n) fc -> q (n fc)", n=N1)[:, g0 * FC:(g0 + gn) * FC],
                        in_=STG[b][:, g0 * FC:(g0 + gn) * FC])

                for f in range(FIL):
                    if f in (1, 4, 7):
                        g_quarter(f // 3 + 1)
                    cmul_m3_t2(1, f)
                pend = []
                done1 = 0
                for f in range(FIL):
                    cmul_m3_t2(0, f)
                    # retire deferred b1 evacs (emitted 2 pairs after their MMs)
                    while pend and pend[0][0] <= f - 2:
                        _, g0, yp_t, eng = pend.pop(0)
                        m4_evac(1, g0, yp_t, 4, eng)
                        done1 += 1
                        if done1 == 8:
                            out_chunk(1, 0)
                        elif done1 == 12:
                            out_chunk(1, 16)
                    if f >= 8:
                        for g in range(3):
                            gi = (f - 8) * 3 + g
                            if gi < 16:
                                eng = "dve" if gi % 2 == 0 else "act"
                                yp_t = m4_mm(1, gi * 4, 4, seeded=(eng == "act"))
                                pend.append((f, gi * 4, yp_t, eng))
                for _, g0, yp_t, eng in pend:
                    m4_evac(1, g0, yp_t, 4, eng)
                out_chunk(1, 32)
                out_chunk(1, 48)
                for g0 in range(0, N1, 4):
                    m4_group(0, g0)
                    if g0 % 16 == 12:
                        out_chunk(0, g0 - 12)

    nc.compile()
    return nc


def host_inputs(cfg, x_sh, w_real, w_imag, s, b):
    """Build the per-core in_map (numpy) for one core's batch shard."""
    import ml_dtypes
    cs = host_consts(cfg)
    N1, N2, FIL, C, FC, BL = cfg.N1, cfg.N2, cfg.FIL, cfg.C, cfg.FC, cfg.BL
    f32 = np.float32
    x_sh = np.asarray(x_sh, f32)
    xs = np.ascontiguousarray(
        x_sh.reshape(BL, N2, N1, C).transpose(1, 0, 2, 3)).reshape(N2, BL * N1 * C)
    wr = np.ascontiguousarray(
        np.asarray(w_real, f32).reshape(FIL, N2, N1).transpose(1, 0, 2)).reshape(N2, FIL * N1)
    wi = np.ascontiguousarray(
        np.asarray(w_imag, f32).reshape(FIL, N2, N1).transpose(1, 0, 2)).reshape(N2, FIL * N1)
    blob_f = cs["blob_f"].copy()
    blob_f[:, 3 * N1:] = np.broadcast_to(np.asarray(b, f32).reshape(1, FC), (N2, FC))
    return {
        "xs": xs, "wr": wr, "wi": wi,
        "s64": np.broadcast_to(np.asarray(s, f32).reshape(1, FIL), (N1, FIL)).copy(),
        "brow": np.asarray(b, f32).reshape(1, FC).astype(ml_dtypes.bfloat16),
        "onesr": np.ones((1, N2), f32).astype(ml_dtypes.bfloat16),
        "blob_r": cs["blob_r"],
        "blob_f": blob_f,
        "blob_b": cs["blob_b"].astype(ml_dtypes.bfloat16),
        "c_L": cs["c_L"].astype(ml_dtypes.bfloat16),
    }


_NC_CACHE = {}


def kernel(x, w_real, w_imag, s, b):
    """Full-input entry point: shard over 8 cores, run, gather."""
    from concourse.bass_utils import run_bass_kernel_spmd
    cfg = FULL
    n_cores = 8
    if "full" not in _NC_CACHE:
        _NC_CACHE["full"] = build_nc(cfg)
    nc = _NC_CACHE["full"]
    x = np.asarray(x, dtype=np.float32)
    in_maps = [host_inputs(cfg, x[i * cfg.BL:(i + 1) * cfg.BL], w_real, w_imag, s, b)
               for i in range(n_cores)]
    res = run_bass_kernel_spmd(nc, in_maps, core_ids=list(range(n_cores)))
    outs = [np.asarray(res.results[i]["out"]).astype(np.float32) for i in range(n_cores)]
    return np.concatenate(outs, axis=0)



# revision 20
# speedup vs baseline: 1.0523x; 1.0523x over previous
"""Trainium2 Bass kernel v6: packed-complex Wiener deconvolution.

v6 over v5: even/odd channel pairs packed as complex rows (halves the
spectrum-multiply, M3 matmuls, T2 transposes, and evac traffic); filter
spectrum Hermitian-ized (g~ = (g[k]+conj(g[-k]))/2) via a P,Q dual-DFT with
sign-baked +/- M2 weight sets (no conj-flip indexing); |h|^2 pair-sum and the
+s regularizer folded into PE matmuls; bias enters through the M3 DC bin as a
rank-1 PE seed; M4 uses 3 cL components (Lr, -Li, +Li); the whole H->G chain
is quarter-pipelined across PE/ACT/DVE; Pool handles the (SBUF-only) x-twiddle
and part of the spectrum multiplies.
"""
import sys

sys.path.insert(0, "/opt/trn_rl_repo")

import numpy as np


def _get_cc():
    import concourse.bacc as bacc
    import concourse.mybir as mybir
    import concourse.tile as tile
    return bacc, mybir, tile


class Cfg:
    def __init__(self, T=8192, N2=128, N1=64, BL=2, C=8, FIL=16):
        assert N1 * N2 == T
        self.T, self.N2, self.N1, self.BL, self.C, self.FIL = T, N2, N1, BL, C, FIL
        self.CP = C // 2
        self.FC = FIL * C


FULL = Cfg()


def host_consts(cfg):
    T, N1, N2, FIL, CP = cfg.T, cfg.N1, cfg.N2, cfg.FIL, cfg.CP
    f32 = np.float32
    n2a, n1a, k2a, k1a = (np.arange(N2), np.arange(N1), np.arange(N2), np.arange(N1))
    cs = {}
    F2 = np.exp(-2j * np.pi * np.outer(n2a, k2a) / N2)          # [n2,k2]
    cs["blob_r"] = np.concatenate(
        [F2.real, F2.imag, -F2.imag], axis=1).astype(f32)
    Tw = np.exp(-2j * np.pi * np.outer(k2a, n1a) / T)           # [k2,n1]
    tw3 = np.concatenate([Tw.real, Tw.imag, -Tw.imag], axis=1).astype(f32)
    cs["blob_twb"] = tw3                                        # ->bf16 twiddles
    F1 = np.exp(-2j * np.pi * np.outer(n1a, k1a) / N1)          # [n1,k1]
    Wstd = np.vstack([np.hstack([F1.real, F1.imag]),
                      np.hstack([-F1.imag, F1.real])]).astype(f32)  # [n1s,k1s]
    Wre, Wim = Wstd[:, :N1], Wstd[:, N1:]
    # set+ out rows: [h+r; -h+i] = [Pr - Qi; -(Pi + Qr)]
    Wp_p = np.hstack([Wre, -Wim])
    Wq_p = np.hstack([-Wim, -Wre])
    # set- out rows: [h-r; h-i] = [Pr + Qi; -Pi + Qr]
    Wp_m = np.hstack([Wre, -Wim])
    Wq_m = np.hstack([Wim, Wre])
    cs["blob_m2"] = np.concatenate([Wstd, Wp_p, Wq_p, Wp_m, Wq_m], axis=1)
    F1b = np.exp(+2j * np.pi * np.outer(k1a, n1a) / N1)         # [k1,j]
    Fbr, Fbi = F1b.real, F1b.imag
    M3A = np.hstack([np.vstack([Fbr, -Fbi]), np.vstack([Fbi, Fbr])]) * 0.5
    M3B = np.hstack([np.vstack([-Fbi, -Fbr]), np.vstack([Fbr, -Fbi])]) * 0.5
    cs["blob_m3"] = np.concatenate([M3A, M3B], axis=1).astype(f32)
    I64 = np.eye(N1, dtype=f32)
    Spair = np.vstack([I64, I64])
    cs["blob_sel"] = np.hstack([Spair, Spair]).astype(f32)      # [k1s, 128]
    ia = np.arange(N2)
    L = np.exp(2j * np.pi * (np.outer(k2a, ia * N1)[:, None, :]
                             + k2a[:, None, None] * n1a[None, :, None]) / T) / T
    cs["cLAB"] = np.concatenate(
        [L.real.reshape(N2, N1 * N2), -L.imag.reshape(N2, N1 * N2),
         L.imag.reshape(N2, N1 * N2)],
        axis=1).astype(f32)                                     # [k2,(n1p,i)x3]
    return cs


def build_nc(cfg):
    bacc, mybir, tile = _get_cc()
    F32, F32R, BF16 = mybir.dt.float32, mybir.dt.float32r, mybir.dt.bfloat16
    AL = mybir.AluOpType
    T, N2, N1, BL, C, FIL, CP = (cfg.T, cfg.N2, cfg.N1, cfg.BL, cfg.C,
                                 cfg.FIL, cfg.CP)
    FC = cfg.FC
    N1s = 2 * N1                  # 128
    KF = FIL * N2                 # 2048
    HN = FIL * N1                 # 1024
    XN = BL * N1 * C              # 1024 (dram x layout, c innermost)
    CK = CP * N2                  # 512
    FH = FIL // 2                 # 8 filters per psum half-batch
    NF4 = FIL // 4                # 4 filters per G quarter
    QW = KF // 4                  # 512
    MCH = 512

    nc = bacc.Bacc("TRN2", debug=False)

    xs_d = nc.dram_tensor("xs", [N2, XN], F32R, kind="ExternalInput")
    wr_d = nc.dram_tensor("wr", [N2, HN], F32R, kind="ExternalInput")
    wi_d = nc.dram_tensor("wi", [N2, HN], F32R, kind="ExternalInput")
    blob_r_d = nc.dram_tensor("blob_r", [N2, 3 * N2], F32R, kind="ExternalInput")
    blob_twb_d = nc.dram_tensor("blob_twb", [N2, 3 * N1], BF16, kind="ExternalInput")
    blob_m2_d = nc.dram_tensor("blob_m2", [N1s, 5 * N1s], BF16, kind="ExternalInput")
    blob_m3_d = nc.dram_tensor("blob_m3", [N1s, 2 * N1s], BF16, kind="ExternalInput")
    blob_sel_d = nc.dram_tensor("blob_sel", [N1s, N1s], BF16, kind="ExternalInput")
    seeds_d = nc.dram_tensor("seeds", [1, KF + 3 * N1s + 2 * FIL * CP], BF16,
                             kind="ExternalInput")
    cLAB_d = nc.dram_tensor("cLAB", [N2, 3 * N1 * N2], BF16, kind="ExternalInput")
    out_d = nc.dram_tensor("out", [BL, T, FC], BF16, kind="ExternalOutput")

    with tile.TileContext(nc) as tc:
        from contextlib import ExitStack
        with tc.tile_pool(name="consts", bufs=1) as cpool, \
             tc.tile_pool(name="pers", bufs=1) as pers:
            # ---------- loads (SP queue, in order) ----------
            def load(name, shape, dt, dram):
                t = cpool.tile(shape, dt, tag=name, name=name)
                nc.sync.dma_start(out=t, in_=dram.ap())
                return t

            blob_r = load("blob_r", [N2, 3 * N2], F32R, blob_r_d)
            wtr = load("wtr", [N2, HN], F32R, wr_d)
            wti = load("wti", [N2, HN], F32R, wi_d)
            blob_twb = load("blob_twb", [N2, 3 * N1], BF16, blob_twb_d)
            blob_m2 = load("blob_m2", [N1s, 5 * N1s], BF16, blob_m2_d)
            blob_m3 = load("blob_m3", [N1s, 2 * N1s], BF16, blob_m3_d)
            blob_sel = load("blob_sel", [N1s, N1s], BF16, blob_sel_d)
            seeds = load("seeds", [1, KF + 3 * N1s + 2 * FIL * CP], BF16, seeds_d)
            xt = load("xt", [N2, XN], F32R, xs_d)
            cLAB = cpool.tile([N2, 3 * N1 * N2], BF16, tag="cLAB")
            QL = N1 * N2 // 2                                   # 4096
            for qc in range(6):
                eng = nc.vector if qc % 2 == 0 else nc.scalar
                eng.dma_start(out=cLAB[:, qc * QL:(qc + 1) * QL],
                              in_=cLAB_d.ap()[:, qc * QL:(qc + 1) * QL])

            F2r = blob_r[:, 0:N2]
            F2i = blob_r[:, N2:2 * N2]
            F2in = blob_r[:, 2 * N2:3 * N2]
            twrb = blob_twb[:, 0:N1]
            twib = blob_twb[:, N1:2 * N1]
            twinb = blob_twb[:, 2 * N1:3 * N1]
            cM2x = blob_m2[:, 0:N1s]
            Wp_p = blob_m2[:, N1s:2 * N1s]
            Wq_p = blob_m2[:, 2 * N1s:3 * N1s]
            Wp_m = blob_m2[:, 3 * N1s:4 * N1s]
            Wq_m = blob_m2[:, 4 * N1s:5 * N1s]
            cM3A = blob_m3[:, 0:N1s]
            cM3B = blob_m3[:, N1s:2 * N1s]
            selA = blob_sel[:, 0:N1s]
            srow = seeds[:, 0:KF]
            ones1 = seeds[:, KF:KF + N1s]
            cselRe = seeds[:, KF + N1s:KF + 2 * N1s]
            cselIm = seeds[:, KF + 2 * N1s:KF + 3 * N1s]
            seedRe = seeds[:, KF + 3 * N1s:KF + 3 * N1s + FIL * CP]
            seedIm = seeds[:, KF + 3 * N1s + FIL * CP:]
            cLA = cLAB[:, 0:N1 * N2]
            cLB = cLAB[:, N1 * N2:2 * N1 * N2]
            cLC = cLAB[:, 2 * N1 * N2:3 * N1 * N2]

            # persistent tiles
            Z0A = pers.tile([N1s, BL * CK], BF16, tag="Z0A")     # [k1s,(b,cp,k2)]
            Grep = pers.tile([N1s, FIL * 2 * N2], BF16, tag="Grep")
            Grv = Grep.rearrange("p (f m q) -> p f m q", f=FIL, m=2)
            hpm = pers.tile([N1s, KF], BF16, tag="hpm")          # [h+r; -h+i]
            hmm = pers.tile([N1s, KF], BF16, tag="hmm")          # [h-r; h-i]
            sqp = pers.tile([N1s, KF], BF16, tag="sqp")
            sqm = pers.tile([N1s, KF], BF16, tag="sqm")
            RP = pers.tile([N1s, KF], BF16, tag="RP")            # r+ both halves
            RM = pers.tile([N1s, KF], BF16, tag="RM")            # r- both halves
            G12s = pers.tile([N1s, QW], BF16, tag="G12s")

            fes = ExitStack()
            fwd = fes.enter_context(tc.tile_pool(name="fwd", bufs=1))
            pxes = ExitStack()
            pAx = pxes.enter_context(tc.tile_pool(name="pAx", bufs=1, space="PSUM"))
            phes = ExitStack()
            pAh = phes.enter_context(tc.tile_pool(name="pAh", bufs=1, space="PSUM"))

            # ---------- M1: H half 0, then x, then H half 1 ----------
            SH = fwd.tile([N2, 2 * 2 * FIL * N1], BF16, tag="SH")
            SHv = SH.rearrange("p (g m f n) -> p g m f n", g=2, m=2, f=FIL)
            SX = fwd.tile([N2, 2 * BL * CP * N1], BF16, tag="SX")
            SXv = SX.rearrange("p (m b c n) -> p m b c n", m=2, b=BL, c=CP)
            wtrv = wtr.rearrange("p (f n) -> p f n", f=FIL)
            wtiv = wti.rearrange("p (f n) -> p f n", f=FIL)
            Ax = pAx.tile([N2, 2 * BL * CP * N1], F32, tag="Ax")  # [k2,(m,b,cp,n1)]
            Axv = Ax.rearrange("p (m b c n) -> p m b c n", m=2, b=BL, c=CP)
            xtv = xt.rearrange("p (b n c e) -> p b n c e", b=BL, n=N1, c=CP)
            xe = xtv[:, :, :, :, 0].transpose([0, 1, 3, 2])      # [n2,(b,cp,n1)]
            xo = xtv[:, :, :, :, 1].transpose([0, 1, 3, 2])

            def m1h_half(h):
                Ahh = pAh.tile([N2, 2 * 2 * FH * N1], F32, tag="Ah", name=f"Ah{h}")
                Av = Ahh.rearrange("p (g m f n) -> p g m f n", g=2, m=2, f=FH)
                fsl = slice(h * FH, (h + 1) * FH)
                nc.tensor.matmul(Av[:, 0, 0], F2r, wtrv[:, fsl], start=True, stop=True)
                nc.tensor.matmul(Av[:, 0, 1], F2i, wtrv[:, fsl], start=True, stop=True)
                nc.tensor.matmul(Av[:, 1, 0], F2r, wtiv[:, fsl], start=True, stop=True)
                nc.tensor.matmul(Av[:, 1, 1], F2i, wtiv[:, fsl], start=True, stop=True)
                nc.scalar.copy(out=SHv[:, 0, :, fsl, :], in_=Av[:, 0])
                nc.scalar.copy(out=SHv[:, 1, :, fsl, :], in_=Av[:, 1])

            m1h_half(0)
            nc.tensor.matmul(Axv[:, 0], F2r, xe, start=True, stop=False)
            nc.tensor.matmul(Axv[:, 0], F2in, xo, start=False, stop=True)
            nc.tensor.matmul(Axv[:, 1], F2i, xe, start=True, stop=False)
            nc.tensor.matmul(Axv[:, 1], F2r, xo, start=False, stop=True)
            nc.scalar.copy(out=SX, in_=Ax)
            m1h_half(1)

            # ---------- H twiddle (DVE, bf16 2x) ----------
            Bh = fwd.tile([N2, 2 * FIL * 2 * N1], BF16, tag="Bh")
            Bhv = Bh.rearrange("p (h g f m n) -> p h g f m n", h=2, g=2, f=FH, m=2)
            uh = fwd.tile([N2, FH * N1], BF16, tag="uh")
            vh = fwd.tile([N2, FH * N1], BF16, tag="vh")
            uhv = uh.rearrange("p (f n) -> p f n", f=FH)
            vhv = vh.rearrange("p (f n) -> p f n", f=FH)

            def bch(w):
                return w[:, None, :].broadcast_to([N2, FH, N1])

            for h in range(2):        # f-half outer so T1-H can go per half
                fsl = slice(h * FH, (h + 1) * FH)
                for g in range(2):    # P, Q
                    nc.vector.tensor_tensor(out=uhv, in0=SHv[:, g, 0, fsl, :],
                                            in1=bch(twrb), op=AL.mult)
                    nc.vector.tensor_tensor(out=vhv, in0=SHv[:, g, 1, fsl, :],
                                            in1=bch(twinb), op=AL.mult)
                    nc.vector.tensor_tensor(out=Bhv[:, h, g, :, 0, :], in0=uhv,
                                            in1=vhv, op=AL.add)
                    nc.vector.tensor_tensor(out=uhv, in0=SHv[:, g, 0, fsl, :],
                                            in1=bch(twib), op=AL.mult)
                    nc.vector.tensor_tensor(out=vhv, in0=SHv[:, g, 1, fsl, :],
                                            in1=bch(twrb), op=AL.mult)
                    nc.vector.tensor_tensor(out=Bhv[:, h, g, :, 1, :], in0=uhv,
                                            in1=vhv, op=AL.add)

            # ---------- x twiddle (Pool, staged bf16 SBUF) ----------
            Bc = fwd.tile([N2, BL * CP * 2 * N1], BF16, tag="Bc")
            Bcv = Bc.rearrange("p (b c m n) -> p b c m n", b=BL, c=CP, m=2)
            ux = fwd.tile([N2, CP * N1], BF16, tag="ux")
            vx = fwd.tile([N2, CP * N1], BF16, tag="vx")
            uxv = ux.rearrange("p (c n) -> p c n", c=CP)
            vxv = vx.rearrange("p (c n) -> p c n", c=CP)

            def bcx(w):
                return w[:, None, :].broadcast_to([N2, CP, N1])

            for b in range(BL):
                nc.gpsimd.tensor_tensor(out=uxv, in0=SXv[:, 0, b], in1=bcx(twrb),
                                        op=AL.mult)
                nc.gpsimd.tensor_tensor(out=vxv, in0=SXv[:, 1, b], in1=bcx(twinb),
                                        op=AL.mult)
                nc.gpsimd.tensor_tensor(out=Bcv[:, b, :, 0, :], in0=uxv, in1=vxv,
                                        op=AL.add)
                nc.gpsimd.tensor_tensor(out=uxv, in0=SXv[:, 0, b], in1=bcx(twib),
                                        op=AL.mult)
                nc.gpsimd.tensor_tensor(out=vxv, in0=SXv[:, 1, b], in1=bcx(twrb),
                                        op=AL.mult)
                nc.gpsimd.tensor_tensor(out=Bcv[:, b, :, 1, :], in0=uxv, in1=vxv,
                                        op=AL.add)

            # ---------- T1 transposes (per H half, then x) ----------
            BTH = fwd.tile([N1s, 2 * 2 * FH * N2], BF16, tag="BTH")
            BTHg = BTH.rearrange("p (h g f q) -> p h g f q", h=2, g=2, f=FH)
            for h in range(2):
                nc.sync.dma_start_transpose(
                    out=BTHg[:, h].rearrange("p g f q -> p (g f) q"),
                    in_=Bhv[:, h].rearrange("p g f m n -> p (g f) (m n)"))
            BTx = fwd.tile([N1s, BL * CP * N2], BF16, tag="BTx")   # [n1s,(b,cp,k2)]
            nc.sync.dma_start_transpose(
                out=BTx.rearrange("p (b c q) -> p (b c) q", b=BL, c=CP), in_=Bc)

            # ---------- quarter-pipelined M2h/squares/SS/recip + M2x ----------
            phes.close()
            pxes.close()
            pZes = ExitStack()
            pZ = pZes.enter_context(tc.tile_pool(name="pZ", bufs=1, space="PSUM"))
            pHes = ExitStack()
            pH = pHes.enter_context(tc.tile_pool(name="pH", bufs=3, space="PSUM"))
            pSes = ExitStack()
            pS = pSes.enter_context(tc.tile_pool(name="pS", bufs=2, space="PSUM"))

            def m2h_q(q):
                """Quarter q of both +/- sets -> hpm/hmm + squares."""
                qs = slice(q * QW, (q + 1) * QW)
                h, fl = q // 2, slice((q % 2) * NF4, (q % 2 + 1) * NF4)
                for (Wp_, Wq_, dsth, dstsq, nm) in (
                        (Wp_p, Wq_p, hpm, sqp, "p"), (Wp_m, Wq_m, hmm, sqm, "m")):
                    Hq = pH.tile([N1s, QW], F32, tag="Hq", name=f"Hq{nm}{q}")
                    nc.tensor.matmul(Hq, Wp_, BTHg[:, h, 0, fl, :], start=True,
                                     stop=False)
                    nc.tensor.matmul(Hq, Wq_, BTHg[:, h, 1, fl, :], start=False,
                                     stop=True)
                    nc.scalar.square(dstsq[:, qs], Hq)
                    nc.scalar.copy(out=dsth[:, qs], in_=Hq)

            def ss_q(q):
                qs = slice(q * QW, (q + 1) * QW)
                for (sqt, rrt, nm) in ((sqp, RP, "p"), (sqm, RM, "m")):
                    SSq = pS.tile([N1s, QW], F32, tag="SSq", name=f"SS{nm}{q}")
                    nc.tensor.matmul(SSq, selA, sqt[:, qs], start=True, stop=False)
                    nc.tensor.matmul(SSq, ones1, srow[:, qs], start=False, stop=True)
                    with nc.allow_low_precision(reason="bf16 wiener gain"):
                        nc.vector.reciprocal(out=rrt[:, qs], in_=SSq)

            def g_quarter(q):
                # G12 rows: [G1(k1); G2(k1)] = hpm*RP + hmm*RM (all aligned)
                qs = slice(q * QW, (q + 1) * QW)
                fq = slice(q * NF4, (q + 1) * NF4)
                nc.vector.tensor_tensor(out=G12s, in0=hpm[:, qs], in1=RP[:, qs],
                                        op=AL.mult)
                hmv = hmm.rearrange("p (f q) -> p f q", f=FIL)[:, fq, :]
                rmv = RM.rearrange("p (f q) -> p f q", f=FIL)[:, fq, :]
                nc.vector.tensor_tensor(out=Grv[:, fq, 0, :], in0=hmv, in1=rmv,
                                        op=AL.mult)
                g12v = G12s.rearrange("p (f q) -> p f q", f=NF4)
                nc.vector.tensor_tensor(out=Grv[:, fq, 0, :],
                                        in0=Grv[:, fq, 0, :], in1=g12v, op=AL.add)
                # rows now [G1; G2] in slot m=0; scatter to (m, halves)
                nc.vector.tensor_copy(out=Grv[:N1, fq, 1, :], in_=Grv[N1:, fq, 0, :])
                nc.vector.tensor_copy(out=Grv[N1:, fq, 1, :], in_=Grv[N1:, fq, 0, :])
                nc.vector.tensor_copy(out=Grv[N1:, fq, 0, :], in_=Grv[:N1, fq, 0, :])

            # software-pipelined emission: PE one stage ahead of evac deps
            m2h_q(0)
            m2h_q(1)
            ss_q(0)
            m2h_q(2)
            ss_q(1)
            # M2x (waits T1x) + Z0A evacs on DVE
            Zps = pZ.tile([N1s, BL * CK], F32, tag="Zps")
            for b in range(BL):
                bsl = slice(b * CK, (b + 1) * CK)
                nc.tensor.matmul(Zps[:, bsl], cM2x, BTx[:, bsl], start=True,
                                 stop=True)
            m2h_q(3)
            ss_q(2)
            ss_q(3)
            g_quarter(0)
            for b in range(BL):
                bsl = slice(b * CK, (b + 1) * CK)
                nc.vector.tensor_copy(out=Z0A[:, bsl], in_=Zps[:, bsl])
            pSes.close()
            pHes.close()
            pZes.close()
            fes.close()

            # ================= inverse =================
            zvA = Z0A.rearrange("p (b c q) -> p b c q", b=BL, c=CP)
            with tc.tile_pool(name="dt", bufs=1) as dtp, \
                 tc.tile_pool(name="stg", bufs=1) as stp, \
                 tc.tile_pool(name="zt", bufs=3) as ztp, \
                 tc.tile_pool(name="cse", bufs=2) as csp, \
                 tc.tile_pool(name="ddp", bufs=2, space="PSUM") as ddp, \
                 tc.tile_pool(name="yp", bufs=4, space="PSUM") as yps:
                DT0 = dtp.tile([N2, FIL * CP * N1s], BF16, tag="DT0")
                DT1 = dtp.tile([N2, FIL * CP * N1s], BF16, tag="DT1")
                DT = [DT0, DT1]                       # [k2,(f,cp,m,n1')]
                STG0 = stp.tile([N2, N1 * FC], BF16, tag="STG0")
                STG1 = stp.tile([N2, N1 * FC], BF16, tag="STG1")
                STG = [STG0, STG1]                    # [i,(j,f,cp,m)]
                CSEQ = [csp.tile([N1s, 4 * CK], BF16, tag=f"cseq{i}",
                                 name=f"cseq{i}") for i in range(2)]

                POOL_ZT = {(1, 2), (1, 5), (1, 8), (1, 11), (1, 14),
                           (0, 1), (0, 4), (0, 7), (0, 10), (0, 13)}

                def cmul_m3(b, f, cseq):
                    """zt12 -> DD -> cse (into quad buffer slot f%4)."""
                    zt = ztp.tile([N1s, 2 * CK], BF16, tag="zt")
                    ztv = zt.rearrange("p (m c q) -> p m c q", m=2, c=CP)
                    g12 = Grv[:, f][:, :, None, :].broadcast_to([N1s, 2, CP, N2])
                    zin = zvA[:, b][:, None, :, :].broadcast_to([N1s, 2, CP, N2])
                    eng = nc.gpsimd if (b, f) in POOL_ZT else nc.vector
                    eng.tensor_tensor(out=ztv, in0=zin, in1=g12, op=AL.mult)
                    DD = ddp.tile([N1s, CK], F32, tag="DD")
                    nc.tensor.matmul(DD, cM3A, zt[:, :CK], start=True, stop=False)
                    nc.tensor.matmul(DD, cM3B, zt[:, CK:], start=False, stop=False)
                    DDv = DD.rearrange("p (c q) -> p c q", c=CP)
                    srv = seedRe.rearrange("o (f c) -> o f c", f=FIL)
                    siv = seedIm.rearrange("o (f c) -> o f c", f=FIL)
                    nc.tensor.matmul(DDv[:, :, 0:1], cselRe,
                                     srv[:, f, :, None], start=False, stop=False)
                    nc.tensor.matmul(DDv[:, :, 0:1], cselIm,
                                     siv[:, f, :, None], start=False, stop=True)
                    dst = cseq[:, (f % 4) * CK:(f % 4 + 1) * CK]
                    if f % 2 == 0:
                        nc.scalar.copy(out=dst, in_=DD)
                    else:
                        nc.vector.tensor_copy(out=dst, in_=DD)

                def quad_t2(b, qf, cseq):
                    dtv = DT[b].rearrange("p (f c n) -> p (f c) n", f=FIL, c=CP)
                    nc.sync.dma_start_transpose(
                        out=dtv[:, qf * 16:(qf + 1) * 16, :], in_=cseq)

                def m4_group(b, g0, eng="act"):
                    """4 n1p values; ypsum [i,(j,m,fc64)]."""
                    dtm = DT[b].rearrange("p (f c m n) -> p n m f c",
                                          f=FIL, c=CP, m=2)
                    ypsum = yps.tile([N2, 4 * 2 * N1], F32, tag="yps")
                    ypv = ypsum.rearrange("p (j m o) -> p j m o", j=4, m=2)
                    for j in range(4):
                        n1p = g0 + j
                        wA = cLA[:, n1p * N2:(n1p + 1) * N2]
                        wB = cLB[:, n1p * N2:(n1p + 1) * N2]
                        wC = cLC[:, n1p * N2:(n1p + 1) * N2]
                        dr = dtm[:, n1p, 0]
                        di = dtm[:, n1p, 1]
                        nc.tensor.matmul(ypv[:, j, 0], wA, dr, start=True, stop=False)
                        nc.tensor.matmul(ypv[:, j, 0], wB, di, start=False, stop=True)
                        nc.tensor.matmul(ypv[:, j, 1], wA, di, start=True, stop=False)
                        nc.tensor.matmul(ypv[:, j, 1], wC, dr, start=False, stop=True)
                    dst = STG[b].rearrange("p (n f c m) -> p n f c m",
                                           n=N1, f=FIL, c=CP)[:, g0:g0 + 4]
                    src = ypv.rearrange("p j m (f c) -> p j f c m", f=FIL)
                    if eng == "act":
                        nc.scalar.copy(out=dst, in_=src)
                    else:
                        nc.vector.tensor_copy(out=dst, in_=src)

                def out_chunk(b, g0, gn=16):
                    nc.scalar.dma_start(
                        out=out_d.ap()[b].rearrange(
                            "(q n) fc -> q (n fc)", n=N1)[:, g0 * FC:(g0 + gn) * FC],
                        in_=STG[b][:, g0 * FC:(g0 + gn) * FC])

                EV = ["act", "dve"]
                # ----- loop 1: b=1 M3 -----
                for f in range(FIL):
                    if f in (2, 5, 8):
                        g_quarter(f // 3 + 1)
                    cmul_m3(1, f, CSEQ[(f // 4) % 2])
                    if f % 4 == 3:
                        quad_t2(1, f // 4, CSEQ[(f // 4) % 2])
                # ----- loop 2: b=0 M3 + b=1 M4 -----
                done1 = 0
                for f in range(FIL):
                    cmul_m3(0, f, CSEQ[(f // 4) % 2])
                    if f % 4 == 3:
                        quad_t2(0, f // 4, CSEQ[(f // 4) % 2])
                    if f >= 2:
                        m4_group(1, (f - 2) * 4, EV[f % 2])
                        done1 += 1
                        if done1 % 4 == 0:
                            out_chunk(1, done1 * 4 - 16)
                for g in range(done1, 16):
                    m4_group(1, g * 4, EV[g % 2])
                    done1 += 1
                    if done1 % 4 == 0:
                        out_chunk(1, done1 * 4 - 16)
                # ----- b=0 M4 tail -----
                for g in range(16):
                    m4_group(0, g * 4, EV[g % 2])
                    if g % 4 == 3:
                        out_chunk(0, g * 4 - 12)

    nc.compile()
    return nc


def host_inputs(cfg, x_sh, w_real, w_imag, s, b):
    """Per-core in_map (numpy) for one core's batch shard. Layout-only on
    inputs; constants precomputed."""
    import ml_dtypes
    cs = host_consts(cfg)
    T, N1, N2, FIL, C, CP, BL = (cfg.T, cfg.N1, cfg.N2, cfg.FIL, cfg.C,
                                 cfg.CP, cfg.BL)
    FC, KF, N1s = cfg.FC, FIL * N2, 2 * N1
    f32, bf16 = np.float32, ml_dtypes.bfloat16
    x_sh = np.asarray(x_sh, f32)
    xs = np.ascontiguousarray(
        x_sh.reshape(BL, N2, N1, C).transpose(1, 0, 2, 3)).reshape(N2, BL * N1 * C)
    wr = np.ascontiguousarray(
        np.asarray(w_real, f32).reshape(FIL, N2, N1).transpose(1, 0, 2)
    ).reshape(N2, FIL * N1)
    wi = np.ascontiguousarray(
        np.asarray(w_imag, f32).reshape(FIL, N2, N1).transpose(1, 0, 2)
    ).reshape(N2, FIL * N1)
    sv = np.asarray(s, f32).reshape(FIL)
    srow = np.repeat(sv, N2)[None, :]                       # [1,(f,k2)]
    ones1 = np.ones((1, N1s), f32)
    cselRe = np.concatenate([np.ones(N1), np.zeros(N1)])[None, :].astype(f32)
    cselIm = np.concatenate([np.zeros(N1), np.ones(N1)])[None, :].astype(f32)
    bv = np.asarray(b, f32).reshape(FIL, C)
    seedRe = (T * bv[:, 0::2]).reshape(1, FIL * CP)
    seedIm = (T * bv[:, 1::2]).reshape(1, FIL * CP)
    seeds = np.concatenate(
        [srow, ones1, cselRe, cselIm, seedRe, seedIm], axis=1)
    return {
        "xs": xs, "wr": wr, "wi": wi,
        "blob_r": cs["blob_r"],
        "blob_twb": cs["blob_twb"].astype(bf16),
        "blob_m2": cs["blob_m2"].astype(bf16),
        "blob_m3": cs["blob_m3"].astype(bf16),
        "blob_sel": cs["blob_sel"].astype(bf16),
        "seeds": seeds.astype(bf16),
        "cLAB": cs["cLAB"].astype(bf16),
    }


_NC_CACHE = {}


def kernel(x, w_real, w_imag, s, b):
    """Full-input entry point: shard over 8 cores, run, gather."""
    from concourse.bass_utils import run_bass_kernel_spmd
    cfg = FULL
    n_cores = 8
    if "full" not in _NC_CACHE:
        _NC_CACHE["full"] = build_nc(cfg)
    nc = _NC_CACHE["full"]
    x = np.asarray(x, dtype=np.float32)
    in_maps = [host_inputs(cfg, x[i * cfg.BL:(i + 1) * cfg.BL], w_real, w_imag, s, b)
               for i in range(n_cores)]
    res = run_bass_kernel_spmd(nc, in_maps, core_ids=list(range(n_cores)))
    outs = [np.asarray(res.results[i]["out"]).astype(np.float32)
            for i in range(n_cores)]
    return np.concatenate(outs, axis=0)


# revision 51
# speedup vs baseline: 1.3678x; 1.2998x over previous
"""Trainium2 Bass kernel v6: packed-complex Wiener deconvolution.

v6 over v5: even/odd channel pairs packed as complex rows (halves the
spectrum-multiply, M3 matmuls, T2 transposes, and evac traffic); filter
spectrum Hermitian-ized (g~ = (g[k]+conj(g[-k]))/2) via a P,Q dual-DFT with
sign-baked +/- M2 weight sets (no conj-flip indexing); |h|^2 pair-sum and the
+s regularizer folded into PE matmuls; bias enters through the M3 DC bin as a
rank-1 PE seed; M4 uses 3 cL components (Lr, -Li, +Li); the whole H->G chain
is quarter-pipelined across PE/ACT/DVE; Pool handles the (SBUF-only) x-twiddle
and part of the spectrum multiplies.
"""
import sys

sys.path.insert(0, "/opt/trn_rl_repo")

import numpy as np


def _get_cc():
    import concourse.bacc as bacc
    import concourse.mybir as mybir
    import concourse.tile as tile
    return bacc, mybir, tile


class Cfg:
    def __init__(self, T=8192, N2=128, N1=64, BL=2, C=8, FIL=16):
        assert N1 * N2 == T
        self.T, self.N2, self.N1, self.BL, self.C, self.FIL = T, N2, N1, BL, C, FIL
        self.CP = C // 2
        self.FC = FIL * C


FULL = Cfg()


def host_consts(cfg):
    T, N1, N2, FIL, CP = cfg.T, cfg.N1, cfg.N2, cfg.FIL, cfg.CP
    f32 = np.float32
    n2a, n1a, k2a, k1a = (np.arange(N2), np.arange(N1), np.arange(N2), np.arange(N1))
    cs = {}
    F2 = np.exp(-2j * np.pi * np.outer(n2a, k2a) / N2)          # [n2,k2]
    cs["blob_r"] = np.concatenate(
        [F2.real, F2.imag, -F2.imag], axis=1).astype(f32)
    Tw = np.exp(-2j * np.pi * np.outer(k2a, n1a) / T)           # [k2,n1]
    tw3 = np.concatenate([Tw.real, Tw.imag, -Tw.imag], axis=1).astype(f32)
    cs["blob_twb"] = tw3                                        # ->bf16 twiddles
    F1 = np.exp(-2j * np.pi * np.outer(n1a, k1a) / N1)          # [n1,k1]
    Wstd = np.vstack([np.hstack([F1.real, F1.imag]),
                      np.hstack([-F1.imag, F1.real])]).astype(f32)  # [n1s,k1s]
    Wre, Wim = Wstd[:, :N1], Wstd[:, N1:]
    # set+ out rows: [h+r; -h+i] = [Pr - Qi; -(Pi + Qr)]
    Wp_p = np.hstack([Wre, -Wim])
    Wq_p = np.hstack([-Wim, -Wre])
    # set- out rows: [h-r; h-i] = [Pr + Qi; -Pi + Qr]
    Wp_m = np.hstack([Wre, -Wim])
    Wq_m = np.hstack([Wim, Wre])
    cs["blob_m2"] = np.concatenate([Wstd, Wp_p, Wq_p, Wp_m, Wq_m], axis=1)
    F1b = np.exp(+2j * np.pi * np.outer(k1a, n1a) / N1)         # [k1,j]
    Fbr, Fbi = F1b.real, F1b.imag
    M3A = np.hstack([np.vstack([Fbr, -Fbi]), np.vstack([Fbi, Fbr])]) * 0.5
    M3B = np.hstack([np.vstack([-Fbi, -Fbr]), np.vstack([Fbr, -Fbi])]) * 0.5
    cs["blob_m3"] = np.concatenate([M3A, M3B], axis=1).astype(f32)
    I64 = np.eye(N1, dtype=f32)
    Spair = np.vstack([I64, I64])
    cs["blob_sel"] = np.hstack([Spair, Spair]).astype(f32)      # [k1s, 128]
    ia = np.arange(N2)
    L = np.exp(2j * np.pi * (np.outer(k2a, ia * N1)[:, None, :]
                             + k2a[:, None, None] * n1a[None, :, None]) / T) / T
    cs["cLAB"] = np.concatenate(
        [L.real.reshape(N2, N1 * N2), -L.imag.reshape(N2, N1 * N2),
         L.imag.reshape(N2, N1 * N2)],
        axis=1).astype(f32)                                     # [k2,(n1p,i)x3]
    return cs


def build_nc(cfg):
    bacc, mybir, tile = _get_cc()
    F32, F32R, BF16 = mybir.dt.float32, mybir.dt.float32r, mybir.dt.bfloat16
    AL = mybir.AluOpType
    T, N2, N1, BL, C, FIL, CP = (cfg.T, cfg.N2, cfg.N1, cfg.BL, cfg.C,
                                 cfg.FIL, cfg.CP)
    FC = cfg.FC
    N1s = 2 * N1                  # 128
    KF = FIL * N2                 # 2048
    HN = FIL * N1                 # 1024
    XN = BL * N1 * C              # 1024 (dram x layout, c innermost)
    CK = CP * N2                  # 512
    FH = FIL // 2                 # 8 filters per psum half-batch
    NF4 = FIL // 4                # 4 filters per G quarter
    QW = KF // 4                  # 512
    MCH = 512

    nc = bacc.Bacc("TRN2", debug=False)

    xs_d = nc.dram_tensor("xs", [N2, XN], F32R, kind="ExternalInput")
    wr_d = nc.dram_tensor("wr", [N2, HN], F32R, kind="ExternalInput")
    wi_d = nc.dram_tensor("wi", [N2, HN], F32R, kind="ExternalInput")
    blob_r_d = nc.dram_tensor("blob_r", [N2, 3 * N2], F32R, kind="ExternalInput")
    blob_twb_d = nc.dram_tensor("blob_twb", [N2, 3 * N1], BF16, kind="ExternalInput")
    blob_m2_d = nc.dram_tensor("blob_m2", [N1s, 5 * N1s], BF16, kind="ExternalInput")
    blob_m3_d = nc.dram_tensor("blob_m3", [N1s, 2 * N1s], BF16, kind="ExternalInput")
    blob_sel_d = nc.dram_tensor("blob_sel", [N1s, N1s], BF16, kind="ExternalInput")
    seeds_d = nc.dram_tensor("seeds", [1, KF + 3 * N1s + 2 * FIL * CP], BF16,
                             kind="ExternalInput")
    cLAB_d = nc.dram_tensor("cLAB", [N2, 3 * N1 * N2], BF16, kind="ExternalInput")
    out_d = nc.dram_tensor("out", [BL, T, FC], BF16, kind="ExternalOutput")

    with tile.TileContext(nc) as tc:
        from contextlib import ExitStack
        with tc.tile_pool(name="consts", bufs=1) as cpool, \
             tc.tile_pool(name="pers", bufs=1) as pers:
            # ---------- loads (SP queue, in order) ----------
            def load(name, shape, dt, dram):
                t = cpool.tile(shape, dt, tag=name, name=name)
                nc.sync.dma_start(out=t, in_=dram.ap())
                return t

            blob_r = load("blob_r", [N2, 3 * N2], F32R, blob_r_d)
            wtr = load("wtr", [N2, HN], F32R, wr_d)
            wti = load("wti", [N2, HN], F32R, wi_d)
            blob_twb = load("blob_twb", [N2, 3 * N1], BF16, blob_twb_d)
            blob_m2 = load("blob_m2", [N1s, 5 * N1s], BF16, blob_m2_d)
            blob_m3 = load("blob_m3", [N1s, 2 * N1s], BF16, blob_m3_d)
            blob_sel = load("blob_sel", [N1s, N1s], BF16, blob_sel_d)
            seeds = load("seeds", [1, KF + 3 * N1s + 2 * FIL * CP], BF16, seeds_d)
            xt = load("xt", [N2, XN], F32R, xs_d)
            cLAB = cpool.tile([N2, 3 * N1 * N2], BF16, tag="cLAB")
            QL = N1 * N2 // 4                                   # 2048

            F2r = blob_r[:, 0:N2]
            F2i = blob_r[:, N2:2 * N2]
            F2in = blob_r[:, 2 * N2:3 * N2]
            twrb = blob_twb[:, 0:N1]
            twib = blob_twb[:, N1:2 * N1]
            twinb = blob_twb[:, 2 * N1:3 * N1]
            cM2x = blob_m2[:, 0:N1s]
            Wp_p = blob_m2[:, N1s:2 * N1s]
            Wq_p = blob_m2[:, 2 * N1s:3 * N1s]
            Wp_m = blob_m2[:, 3 * N1s:4 * N1s]
            Wq_m = blob_m2[:, 4 * N1s:5 * N1s]
            cM3A = blob_m3[:, 0:N1s]
            cM3B = blob_m3[:, N1s:2 * N1s]
            selA = blob_sel[:, 0:N1s]
            srow = seeds[:, 0:KF]
            ones1 = seeds[:, KF:KF + N1s]
            cselRe = seeds[:, KF + N1s:KF + 2 * N1s]
            cselIm = seeds[:, KF + 2 * N1s:KF + 3 * N1s]
            seedRe = seeds[:, KF + 3 * N1s:KF + 3 * N1s + FIL * CP]
            seedIm = seeds[:, KF + 3 * N1s + FIL * CP:]
            cLA = cLAB[:, 0:N1 * N2]
            cLB = cLAB[:, N1 * N2:2 * N1 * N2]
            cLC = cLAB[:, 2 * N1 * N2:3 * N1 * N2]

            # persistent tiles
            Z0A = pers.tile([N1s, BL * CK], BF16, tag="Z0A")     # [k1s,(b,cp,k2)]
            Grep = pers.tile([N1s, FIL * 2 * N2], BF16, tag="Grep")
            Grv = Grep.rearrange("p (f m q) -> p f m q", f=FIL, m=2)
            hpm = pers.tile([N1s, KF], BF16, tag="hpm")          # [h+r; -h+i]
            hmm = pers.tile([N1s, KF], BF16, tag="hmm")          # [h-r; h-i]
            sqp = pers.tile([N1s, KF], BF16, tag="sqp")
            sqm = pers.tile([N1s, KF], BF16, tag="sqm")
            RP = pers.tile([N1s, KF], BF16, tag="RP")            # r+ both halves
            RM = pers.tile([N1s, KF], BF16, tag="RM")            # r- both halves
            G12s = pers.tile([N1s, QW], BF16, tag="G12s")
            SS2p = pers.tile([N1s, QW], BF16, tag="SS2p")
            SS2m = pers.tile([N1s, QW], BF16, tag="SS2m")

            fes = ExitStack()
            fwd = fes.enter_context(tc.tile_pool(name="fwd", bufs=1))
            pxes = ExitStack()
            pAx = pxes.enter_context(tc.tile_pool(name="pAx", bufs=1, space="PSUM"))
            phes = ExitStack()
            pAh = phes.enter_context(tc.tile_pool(name="pAh", bufs=1, space="PSUM"))

            # ---------- M1: H quarter-pipelined, x interleaved ----------
            SH = fwd.tile([N2, 2 * 2 * FIL * N1], BF16, tag="SH")
            SHv = SH.rearrange("p (g m f n) -> p g m f n", g=2, m=2, f=FIL)
            SX = fwd.tile([N2, 2 * BL * CP * N1], BF16, tag="SX")
            SXv = SX.rearrange("p (m b c n) -> p m b c n", m=2, b=BL, c=CP)
            wtrv = wtr.rearrange("p (f n) -> p f n", f=FIL)
            wtiv = wti.rearrange("p (f n) -> p f n", f=FIL)
            Ax = pAx.tile([N2, 2 * BL * CP * N1], F32, tag="Ax")  # [k2,(m,b,cp,n1)]
            Axv = Ax.rearrange("p (m b c n) -> p m b c n", m=2, b=BL, c=CP)
            xtv = xt.rearrange("p (b n c e) -> p b n c e", b=BL, n=N1, c=CP)
            xe = xtv[:, :, :, :, 0].transpose([0, 1, 3, 2])      # [n2,(b,cp,n1)]
            xo = xtv[:, :, :, :, 1].transpose([0, 1, 3, 2])

            def m1h_q(q):
                Ahh = pAh.tile([N2, 2 * 2 * NF4 * N1], F32, tag="Ah", name=f"Ah{q}")
                Av = Ahh.rearrange("p (g m f n) -> p g m f n", g=2, m=2, f=NF4)
                fsl = slice(q * NF4, (q + 1) * NF4)
                nc.tensor.matmul(Av[:, 0, 0], F2r, wtrv[:, fsl], start=True, stop=True)
                nc.tensor.matmul(Av[:, 0, 1], F2i, wtrv[:, fsl], start=True, stop=True)
                nc.tensor.matmul(Av[:, 1, 0], F2r, wtiv[:, fsl], start=True, stop=True)
                nc.tensor.matmul(Av[:, 1, 1], F2i, wtiv[:, fsl], start=True, stop=True)
                nc.scalar.copy(out=SHv[:, 0, :, fsl, :], in_=Av[:, 0])
                nc.scalar.copy(out=SHv[:, 1, :, fsl, :], in_=Av[:, 1])

            m1h_q(0)
            m1h_q(1)
            nc.tensor.matmul(Axv[:, 0], F2r, xe, start=True, stop=False)
            nc.tensor.matmul(Axv[:, 0], F2in, xo, start=False, stop=True)
            nc.tensor.matmul(Axv[:, 1], F2i, xe, start=True, stop=False)
            nc.tensor.matmul(Axv[:, 1], F2r, xo, start=False, stop=True)
            nc.scalar.copy(out=SX, in_=Ax)
            m1h_q(2)
            m1h_q(3)

            # ---------- H twiddle (DVE, bf16 2x), per quarter ----------
            Bh = fwd.tile([N2, 2 * FIL * 2 * N1], BF16, tag="Bh")
            Bhv = Bh.rearrange("p (u g f m n) -> p u g f m n", u=4, g=2, f=NF4, m=2)
            uh = fwd.tile([N2, NF4 * N1], BF16, tag="uh")
            vh = fwd.tile([N2, NF4 * N1], BF16, tag="vh")
            uhv = uh.rearrange("p (f n) -> p f n", f=NF4)
            vhv = vh.rearrange("p (f n) -> p f n", f=NF4)

            def bch(w):
                return w[:, None, :].broadcast_to([N2, NF4, N1])

            BTH = fwd.tile([N1s, 2 * 2 * FH * N2], BF16, tag="BTH")
            BTHg = BTH.rearrange("p (u g f q) -> p u g f q", u=4, g=2, f=NF4)

            uh2 = fwd.tile([N2, NF4 * N1], BF16, tag="uh2")
            vh2 = fwd.tile([N2, NF4 * N1], BF16, tag="vh2")
            uh2v = uh2.rearrange("p (f n) -> p f n", f=NF4)
            vh2v = vh2.rearrange("p (f n) -> p f n", f=NF4)

            def htw_q(u, eng=None, us=None, vs=None):
                eng = eng or nc.vector
                us, vs = us or uhv, vs or vhv
                fsl = slice(u * NF4, (u + 1) * NF4)
                for g in range(2):    # P, Q
                    eng.tensor_tensor(out=us, in0=SHv[:, g, 0, fsl, :],
                                      in1=bch(twrb), op=AL.mult)
                    eng.tensor_tensor(out=vs, in0=SHv[:, g, 1, fsl, :],
                                      in1=bch(twinb), op=AL.mult)
                    eng.tensor_tensor(out=Bhv[:, u, g, :, 0, :], in0=us,
                                      in1=vs, op=AL.mult if False else AL.add)
                    eng.tensor_tensor(out=us, in0=SHv[:, g, 0, fsl, :],
                                      in1=bch(twib), op=AL.mult)
                    eng.tensor_tensor(out=vs, in0=SHv[:, g, 1, fsl, :],
                                      in1=bch(twrb), op=AL.mult)
                    eng.tensor_tensor(out=Bhv[:, u, g, :, 1, :], in0=us,
                                      in1=vs, op=AL.add)

            def t1h_q(u):
                nc.sync.dma_start_transpose(
                    out=BTHg[:, u].rearrange("p g f q -> p (g f) q"),
                    in_=Bhv[:, u].rearrange("p g f m n -> p (g f) (m n)"))

            htw_q(0)
            htw_q(1)
            htw_q(3)

            # ---------- x twiddle (Pool, staged bf16 SBUF), split per b ----------
            Bc = fwd.tile([N2, BL * CP * 2 * N1], BF16, tag="Bc")
            Bcv = Bc.rearrange("p (b c m n) -> p b c m n", b=BL, c=CP, m=2)
            ux = fwd.tile([N2, CP * N1], BF16, tag="ux")
            vx = fwd.tile([N2, CP * N1], BF16, tag="vx")
            uxv = ux.rearrange("p (c n) -> p c n", c=CP)
            vxv = vx.rearrange("p (c n) -> p c n", c=CP)

            def bcx(w):
                return w[:, None, :].broadcast_to([N2, CP, N1])

            def xtw_b(b):
                nc.gpsimd.tensor_tensor(out=uxv, in0=SXv[:, 0, b], in1=bcx(twrb),
                                        op=AL.mult)
                nc.gpsimd.tensor_tensor(out=vxv, in0=SXv[:, 1, b], in1=bcx(twinb),
                                        op=AL.mult)
                nc.gpsimd.tensor_tensor(out=Bcv[:, b, :, 0, :], in0=uxv, in1=vxv,
                                        op=AL.add)
                nc.gpsimd.tensor_tensor(out=uxv, in0=SXv[:, 0, b], in1=bcx(twib),
                                        op=AL.mult)
                nc.gpsimd.tensor_tensor(out=vxv, in0=SXv[:, 1, b], in1=bcx(twrb),
                                        op=AL.mult)
                nc.gpsimd.tensor_tensor(out=Bcv[:, b, :, 1, :], in0=uxv, in1=vxv,
                                        op=AL.add)

            xtw_b(1)
            htw_q(2, eng=nc.gpsimd, us=uh2v, vs=vh2v)
            xtw_b(0)

            # ---------- T1s (SP, ordered by expected readiness) ----------
            BTx = fwd.tile([N1s, BL * CP * N2], BF16, tag="BTx")   # [n1s,(b,cp,k2)]
            BTxv = BTx.rearrange("p (b c q) -> p b c q", b=BL, c=CP)

            def t1x_b(b):
                nc.sync.dma_start_transpose(
                    out=BTxv[:, b].rearrange("p c q -> p c q"),
                    in_=Bcv[:, b].rearrange("p c m n -> p c (m n)"))

            t1h_q(0)
            t1x_b(1)
            t1h_q(1)
            t1h_q(3)
            t1h_q(2)
            t1x_b(0)
            for qc in range(12):
                nc.sync.dma_start(out=cLAB[:, qc * QL:(qc + 1) * QL],
                                  in_=cLAB_d.ap()[:, qc * QL:(qc + 1) * QL])

            # ---------- quarter-pipelined M2h/squares/SS/recip + M2x ----------
            phes.close()
            pxes.close()
            pZes = ExitStack()
            pZ = pZes.enter_context(tc.tile_pool(name="pZ", bufs=1, space="PSUM"))
            pHes = ExitStack()
            pH = pHes.enter_context(tc.tile_pool(name="pH", bufs=3, space="PSUM"))
            pSes = ExitStack()
            pS = pSes.enter_context(tc.tile_pool(name="pS", bufs=2, space="PSUM"))

            def m2h_q(q):
                """Quarter q of both +/- sets -> hpm/hmm + squares."""
                qs = slice(q * QW, (q + 1) * QW)
                for (Wp_, Wq_, dsth, dstsq, nm) in (
                        (Wp_p, Wq_p, hpm, sqp, "p"), (Wp_m, Wq_m, hmm, sqm, "m")):
                    Hq = pH.tile([N1s, QW], F32, tag="Hq", name=f"Hq{nm}{q}")
                    nc.tensor.matmul(Hq, Wp_, BTHg[:, q, 0].rearrange(
                        "p f q -> p (f q)"), start=True, stop=False)
                    nc.tensor.matmul(Hq, Wq_, BTHg[:, q, 1].rearrange(
                        "p f q -> p (f q)"), start=False, stop=True)
                    nc.scalar.square(dstsq[:, qs], Hq)
                    nc.scalar.copy(out=dsth[:, qs], in_=Hq)

            def ss_q(q, stage=False):
                qs = slice(q * QW, (q + 1) * QW)
                for (sqt, rrt, st) in ((sqp, RP, SS2p), (sqm, RM, SS2m)):
                    nm = "p" if sqt is sqp else "m"
                    SSq = pS.tile([N1s, QW], F32, tag="SSq", name=f"SS{nm}{q}")
                    nc.tensor.matmul(SSq, selA, sqt[:, qs], start=True, stop=False)
                    nc.tensor.matmul(SSq, ones1, srow[:, qs], start=False, stop=True)
                    if stage:
                        nc.scalar.copy(out=st, in_=SSq)
                    else:
                        with nc.allow_low_precision(reason="bf16 wiener gain"):
                            nc.vector.reciprocal(out=rrt[:, qs], in_=SSq)

            def recip_q2():
                qs = slice(2 * QW, 3 * QW)
                with nc.allow_low_precision(reason="bf16 wiener gain"):
                    nc.vector.reciprocal(out=RP[:, qs], in_=SS2p)
                    nc.vector.reciprocal(out=RM[:, qs], in_=SS2m)

            def g_quarter(q):
                # G12 rows: [G1(k1); G2(k1)] = hpm*RP + hmm*RM (all aligned)
                qs = slice(q * QW, (q + 1) * QW)
                fq = slice(q * NF4, (q + 1) * NF4)
                nc.vector.tensor_tensor(out=G12s, in0=hpm[:, qs], in1=RP[:, qs],
                                        op=AL.mult)
                hmv = hmm.rearrange("p (f q) -> p f q", f=FIL)[:, fq, :]
                rmv = RM.rearrange("p (f q) -> p f q", f=FIL)[:, fq, :]
                nc.vector.tensor_tensor(out=Grv[:, fq, 0, :], in0=hmv, in1=rmv,
                                        op=AL.mult)
                g12v = G12s.rearrange("p (f q) -> p f q", f=NF4)
                nc.vector.tensor_tensor(out=Grv[:, fq, 0, :],
                                        in0=Grv[:, fq, 0, :], in1=g12v, op=AL.add)
                # rows now [G1; G2] in slot m=0; scatter to (m, halves)
                nc.vector.tensor_copy(out=Grv[:N1, fq, 1, :], in_=Grv[N1:, fq, 0, :])
                nc.vector.tensor_copy(out=Grv[N1:, fq, 1, :], in_=Grv[N1:, fq, 0, :])
                nc.vector.tensor_copy(out=Grv[N1:, fq, 0, :], in_=Grv[:N1, fq, 0, :])

            # software-pipelined emission: PE one stage ahead of evac deps
            Zps = pZ.tile([N1s, BL * CK], F32, tag="Zps")

            def m2x_b(b):
                bsl = slice(b * CK, (b + 1) * CK)
                nc.tensor.matmul(Zps[:, bsl], cM2x, BTx[:, bsl], start=True,
                                 stop=True)
                nc.scalar.copy(out=Z0A[:, bsl], in_=Zps[:, bsl])

            m2h_q(0)
            m2h_q(1)
            ss_q(0)
            m2x_b(1)
            m2h_q(3)
            ss_q(1)
            ss_q(3)
            g_quarter(0)
            m2h_q(2)
            m2x_b(0)
            ss_q(2)
            pSes.close()
            pHes.close()
            pZes.close()
            fes.close()

            # ================= inverse =================
            zvA = Z0A.rearrange("p (b c q) -> p b c q", b=BL, c=CP)
            with tc.tile_pool(name="dt", bufs=1) as dtp, \
                 tc.tile_pool(name="stg", bufs=1) as stp, \
                 tc.tile_pool(name="zt", bufs=4) as ztp, \
                 tc.tile_pool(name="cse", bufs=5) as csp, \
                 tc.tile_pool(name="ddp", bufs=2, space="PSUM") as ddp, \
                 tc.tile_pool(name="yp", bufs=2, space="PSUM") as yps:
                DT0 = dtp.tile([N2, FIL * CP * N1s], BF16, tag="DT0")
                DT1 = dtp.tile([N2, FIL * CP * N1s], BF16, tag="DT1")
                DT = [DT0, DT1]                       # [k2,(f,cp,m,n1')]
                STG0 = stp.tile([N2, N1 * FC], BF16, tag="STG0")
                STG1 = stp.tile([N2, N1 * FC], BF16, tag="STG1")
                STG = [STG0, STG1]                    # [i,(j,f,cp,m)]
                def new_cseq():
                    return csp.tile([N1s, 4 * CK], BF16, tag="cseq", name="cseq")
                srv = seedRe.rearrange("o (f c) -> o f c", f=FIL)
                siv = seedIm.rearrange("o (f c) -> o f c", f=FIL)

                # Pool zt pairs are prefetched one loop-step early
                POOL_ZT = {(1, 2), (1, 5), (0, 2), (0, 5)}

                def zt_mul(b, fp, eng):
                    f0 = 2 * fp
                    zt = ztp.tile([N1s, 2 * 2 * CK], BF16, tag="zt")
                    ztv = zt.rearrange("p (i m c q) -> p i m c q", i=2, m=2, c=CP)
                    g12 = Grv[:, f0:f0 + 2][:, :, :, None, :].broadcast_to(
                        [N1s, 2, 2, CP, N2])
                    zin = zvA[:, b][:, None, None, :, :].broadcast_to(
                        [N1s, 2, 2, CP, N2])
                    eng.tensor_tensor(out=ztv, in0=zin, in1=g12, op=AL.mult)
                    return zt

                PENDING_ZT = {}

                def cmul_m3_pair(b, fp, cseq):
                    """f = 2*fp, 2*fp+1: 8+4 matmuls, one evac (ACT)."""
                    f0 = 2 * fp
                    zt = PENDING_ZT.pop((b, fp), None)
                    if zt is None:
                        zt = zt_mul(b, fp, nc.vector)
                    ztv = zt.rearrange("p (i m c q) -> p i m c q", i=2, m=2, c=CP)
                    DD = ddp.tile([N1s, 2 * CK], F32, tag="DD")
                    for i in range(2):
                        f = f0 + i
                        sl = DD[:, i * CK:(i + 1) * CK]
                        nc.tensor.matmul(sl, cM3A, ztv[:, i, 0].rearrange(
                            "p c q -> p (c q)"), start=True, stop=False)
                        nc.tensor.matmul(sl, cM3B, ztv[:, i, 1].rearrange(
                            "p c q -> p (c q)"), start=False, stop=False)
                        DDv = sl.rearrange("p (c q) -> p c q", c=CP)
                        nc.tensor.matmul(DDv[:, :, 0:1], cselRe,
                                         srv[:, f, :, None], start=False, stop=False)
                        nc.tensor.matmul(DDv[:, :, 0:1], cselIm,
                                         siv[:, f, :, None], start=False, stop=True)
                    dst = cseq[:, (fp % 2) * 2 * CK:(fp % 2 + 1) * 2 * CK]
                    nc.scalar.copy(out=dst, in_=DD)

                def prefetch_pool_zt(b, fp):
                    if (b, fp) in POOL_ZT:
                        PENDING_ZT[(b, fp)] = zt_mul(b, fp, nc.gpsimd)

                def quad_t2(b, qf, cseq):
                    dtv = DT[b].rearrange("p (f c n) -> p (f c) n", f=FIL, c=CP)
                    nc.sync.dma_start_transpose(
                        out=dtv[:, qf * 16:(qf + 1) * 16, :], in_=cseq)

                def pair_t2(b, fp, cseq):
                    dtv = DT[b].rearrange("p (f c n) -> p (f c) n", f=FIL, c=CP)
                    sl = cseq[:, (fp % 2) * 2 * CK:(fp % 2 + 1) * 2 * CK]
                    nc.sync.dma_start_transpose(
                        out=dtv[:, fp * 8:(fp + 1) * 8, :], in_=sl)

                def m4_group8(b, g0, eng="act"):
                    """8 n1p values; ypsum [i,(j8,m,fc64)]; one evac."""
                    dtm = DT[b].rearrange("p (f c m n) -> p n m f c",
                                          f=FIL, c=CP, m=2)
                    ypsum = yps.tile([N2, 8 * 2 * N1], F32, tag="yps")
                    ypv = ypsum.rearrange("p (j m o) -> p j m o", j=8, m=2)
                    for j in range(8):
                        n1p = g0 + j
                        wA = cLA[:, n1p * N2:(n1p + 1) * N2]
                        wB = cLB[:, n1p * N2:(n1p + 1) * N2]
                        wC = cLC[:, n1p * N2:(n1p + 1) * N2]
                        dr = dtm[:, n1p, 0]
                        di = dtm[:, n1p, 1]
                        nc.tensor.matmul(ypv[:, j, 0], wA, dr, start=True, stop=False)
                        nc.tensor.matmul(ypv[:, j, 0], wB, di, start=False, stop=True)
                        nc.tensor.matmul(ypv[:, j, 1], wA, di, start=True, stop=False)
                        nc.tensor.matmul(ypv[:, j, 1], wC, dr, start=False, stop=True)
                    dst = STG[b].rearrange("p (n f c m) -> p n f c m",
                                           n=N1, f=FIL, c=CP)[:, g0:g0 + 8]
                    src = ypv.rearrange("p j m (f c) -> p j f c m", f=FIL)
                    if eng == "act":
                        nc.scalar.copy(out=dst, in_=src)
                    else:
                        nc.vector.tensor_copy(out=dst, in_=src)

                def m4_group8_h(b, g0, fh, eng="act"):
                    """8 n1p values, HALF the filters (fh*8..fh*8+8)."""
                    dtm = DT[b].rearrange("p (f c m n) -> p n m f c",
                                          f=FIL, c=CP, m=2)
                    fsl = slice(fh * FH, (fh + 1) * FH)
                    ypsum = yps.tile([N2, 8 * 2 * N1], F32, tag="yps")
                    ypv = ypsum.rearrange("p (j m o) -> p j m o", j=8, m=2)
                    hw2 = FH * CP
                    for j in range(8):
                        n1p = g0 + j
                        wA = cLA[:, n1p * N2:(n1p + 1) * N2]
                        wB = cLB[:, n1p * N2:(n1p + 1) * N2]
                        wC = cLC[:, n1p * N2:(n1p + 1) * N2]
                        dr = dtm[:, n1p, 0, fsl, :]
                        di = dtm[:, n1p, 1, fsl, :]
                        o0 = ypv[:, j, 0, :hw2]
                        o1 = ypv[:, j, 1, :hw2]
                        nc.tensor.matmul(o0, wA, dr, start=True, stop=False)
                        nc.tensor.matmul(o0, wB, di, start=False, stop=True)
                        nc.tensor.matmul(o1, wA, di, start=True, stop=False)
                        nc.tensor.matmul(o1, wC, dr, start=False, stop=True)
                    dst = STG[b].rearrange("p (n f c m) -> p n f c m",
                                           n=N1, f=FIL, c=CP)[:, g0:g0 + 8, fsl]
                    src_ = ypv[:, :, :, :hw2].rearrange(
                        "p j m (f c) -> p j f c m", f=FH)
                    if eng == "act":
                        nc.scalar.copy(out=dst, in_=src_)
                    else:
                        nc.vector.tensor_copy(out=dst, in_=src_)

                def out_chunk(b, g0, gn=16):
                    nc.scalar.dma_start(
                        out=out_d.ap()[b].rearrange(
                            "(q n) fc -> q (n fc)", n=N1)[:, g0 * FC:(g0 + gn) * FC],
                        in_=STG[b][:, g0 * FC:(g0 + gn) * FC])

                EV = ["dve", "act"]
                # ----- loop 1: b=1 M3 (8 pair-steps) -----
                cseq = new_cseq()
                for fp in range(FIL // 2):
                    if fp in (1, 3, 5):
                        g_quarter((fp + 1) // 2)
                    prefetch_pool_zt(1, fp + 1)
                    cmul_m3_pair(1, fp, cseq)
                    if fp % 2 == 1:
                        quad_t2(1, fp // 2, cseq)
                        cseq = new_cseq()
                prefetch_pool_zt(0, 0)
                # ----- loop 2: b=0 M3 + all 8 b=1 M4 super-groups -----
                for fp in range(FIL // 2):
                    if fp < 6:
                        m4_group8(1, fp * 8, EV[fp % 2])
                        if fp % 2 == 1 and fp >= 3:
                            out_chunk(1, (fp - 3) * 8)
                        prefetch_pool_zt(0, fp + 1)
                        cmul_m3_pair(0, fp, cseq)
                    else:
                        prefetch_pool_zt(0, fp + 1)
                        cmul_m3_pair(0, fp, cseq)
                        m4_group8(1, fp * 8, EV[fp % 2])
                    if fp >= 6:
                        pair_t2(0, fp, cseq)
                        if fp % 2 == 1:
                            cseq = new_cseq()
                    elif fp % 2 == 1:
                        quad_t2(0, fp // 2, cseq)
                        cseq = new_cseq()
                out_chunk(1, 32)
                out_chunk(1, 48)
                # ----- b=0 M4 tail -----
                for g in range(8):
                    m4_group8(0, g * 8, EV[g % 2])
                    if g % 2 == 1 and g < 7:
                        out_chunk(0, g * 8 - 8)
                out_chunk(0, 48, gn=12)
                out_chunk(0, 60, gn=4)

    nc.compile()
    return nc


def host_inputs(cfg, x_sh, w_real, w_imag, s, b):
    """Per-core in_map (numpy) for one core's batch shard. Layout-only on
    inputs; constants precomputed."""
    import ml_dtypes
    cs = host_consts(cfg)
    T, N1, N2, FIL, C, CP, BL = (cfg.T, cfg.N1, cfg.N2, cfg.FIL, cfg.C,
                                 cfg.CP, cfg.BL)
    FC, KF, N1s = cfg.FC, FIL * N2, 2 * N1
    f32, bf16 = np.float32, ml_dtypes.bfloat16
    x_sh = np.asarray(x_sh, f32)
    xs = np.ascontiguousarray(
        x_sh.reshape(BL, N2, N1, C).transpose(1, 0, 2, 3)).reshape(N2, BL * N1 * C)
    wr = np.ascontiguousarray(
        np.asarray(w_real, f32).reshape(FIL, N2, N1).transpose(1, 0, 2)
    ).reshape(N2, FIL * N1)
    wi = np.ascontiguousarray(
        np.asarray(w_imag, f32).reshape(FIL, N2, N1).transpose(1, 0, 2)
    ).reshape(N2, FIL * N1)
    sv = np.asarray(s, f32).reshape(FIL)
    srow = np.repeat(sv, N2)[None, :]                       # [1,(f,k2)]
    ones1 = np.ones((1, N1s), f32)
    cselRe = np.concatenate([np.ones(N1), np.zeros(N1)])[None, :].astype(f32)
    cselIm = np.concatenate([np.zeros(N1), np.ones(N1)])[None, :].astype(f32)
    bv = np.asarray(b, f32).reshape(FIL, C)
    seedRe = (T * bv[:, 0::2]).reshape(1, FIL * CP)
    seedIm = (T * bv[:, 1::2]).reshape(1, FIL * CP)
    seeds = np.concatenate(
        [srow, ones1, cselRe, cselIm, seedRe, seedIm], axis=1)
    return {
        "xs": xs, "wr": wr, "wi": wi,
        "blob_r": cs["blob_r"],
        "blob_twb": cs["blob_twb"].astype(bf16),
        "blob_m2": cs["blob_m2"].astype(bf16),
        "blob_m3": cs["blob_m3"].astype(bf16),
        "blob_sel": cs["blob_sel"].astype(bf16),
        "seeds": seeds.astype(bf16),
        "cLAB": cs["cLAB"].astype(bf16),
    }


_NC_CACHE = {}


def kernel(x, w_real, w_imag, s, b):
    """Full-input entry point: shard over 8 cores, run, gather."""
    from concourse.bass_utils import run_bass_kernel_spmd
    cfg = FULL
    n_cores = 8
    if "full" not in _NC_CACHE:
        _NC_CACHE["full"] = build_nc(cfg)
    nc = _NC_CACHE["full"]
    x = np.asarray(x, dtype=np.float32)
    in_maps = [host_inputs(cfg, x[i * cfg.BL:(i + 1) * cfg.BL], w_real, w_imag, s, b)
               for i in range(n_cores)]
    res = run_bass_kernel_spmd(nc, in_maps, core_ids=list(range(n_cores)))
    outs = [np.asarray(res.results[i]["out"]).astype(np.float32)
            for i in range(n_cores)]
    return np.concatenate(outs, axis=0)
